# revision 1
# baseline (speedup 1.0000x reference)
"""LongcatMoe Trainium2 kernel — expert-parallel sparse MoE across 8 NeuronCores.

Strategy (expert-parallel, per the sharding hint):
  - Host computes the tiny router (fp64 softmax/top-k) and dispatches tokens
    by top-k expert id: core e receives the tokens routed to expert e (padded
    to capacity C=256, capacity factor 1.0; overflow falls back to an exact
    host computation), plus expert e's weights (cast bf16).
  - Each core runs the silu-gated MLP for its expert on its token block:
      y[:, t] = ((silu(Wg.T x_t)) * (Wu.T x_t)).T @ Wd     in [H, C] layout,
    bf16 matmuls with fp32 PSUM accumulation, y output cast bf16.
  - Host combines: out[tok] += gate_weight * y, plus the zero-expert
    (identity) term zero_w[t] * x[t].

DMA design (from trace analysis): everything rides ONE HWDGE ring (SP) in
exact consumption order so the phase-2 wd prefetch never round-robin-steals
bandwidth from the phase-1 weight stream, and transfers are ≥512KB so each
runs at the ~340-400 GB/s operating point (small transfers are
descriptor-dominated at ~100 GB/s and caused 3-4us PE stalls + HAM
re-throttle).  y output rides the ACT ring alone.

Host-side layouts (per-partition contiguous for every device DMA):
  xT  [128, HO, C]          xT[p, ho, t]      = x[idx[t], ho*128+p]
  wgu [IO, 128, 2, HO, 128] wgu[j,p,0,ho,c]   = w_gate[ho*128+p, j*128+c]
                            wgu[j,p,1,ho,c]   = w_up  [ho*128+p, j*128+c]
  wd  [HO/4, 128, 4, IO, 128] wd[q,p,i,io,c]  = w_down[io*128+p, (4q+i)*128+c]
  y   [H, C] bf16 output
"""

import os

import numpy as np
import ml_dtypes

T, H, I, E, Z, TOPK = 1024, 2048, 1024, 8, 8, 4
ROUTED_SCALING = 1.0
N_CORES = 8
P = 128
HO = H // P   # 16
IO = I // P   # 8
C = 256       # per-expert token capacity (capacity factor 1.0; overflow → host)
WDQ = 4       # wd k-slices per DMA transfer
NWARM = 18    # PE warmup matmuls (bridge HAM to first real matmul ~14us)

_PROGRAM = None
LAST_RESULTS = None  # BassKernelResults of the most recent run (for test harness)


def _build_program():
    import concourse.mybir as mybir
    import concourse.tile as tile
    from concourse import bacc

    f32 = mybir.dt.float32
    bf16 = mybir.dt.bfloat16
    SILU = mybir.ActivationFunctionType.Silu

    nc = bacc.Bacc(
        "TRN2",
        target_bir_lowering=False,
        debug=False,
        enable_asserts=False,
        num_devices=N_CORES,
    )
    xT = nc.dram_tensor("xT", [P, HO, C], bf16, kind="ExternalInput").ap()
    wgu = nc.dram_tensor("wgu", [IO, P, 2, HO, P], bf16,
                         kind="ExternalInput").ap()
    wd = nc.dram_tensor("wd", [HO // WDQ, P, WDQ, IO, P], bf16,
                        kind="ExternalInput").ap()
    y = nc.dram_tensor("y", [H, C], bf16, kind="ExternalOutput").ap()

    with tile.TileContext(nc) as tc:
        with (
            tc.tile_pool(name="px", bufs=1) as px,
            tc.tile_pool(name="pwgu", bufs=IO) as pwgu,
            tc.tile_pool(name="pwd", bufs=HO // WDQ) as pwd,
            tc.tile_pool(name="pmid", bufs=IO) as pmid,
            tc.tile_pool(name="psg", bufs=2) as psg,
            tc.tile_pool(name="py", bufs=6) as py,
            tc.tile_pool(name="pwrm", bufs=1) as pwrm,
            tc.tile_pool(name="ppg", bufs=2, space="PSUM") as ppg,
            tc.tile_pool(name="ppu", bufs=2, space="PSUM") as ppu,
            tc.tile_pool(name="ppd", bufs=3, space="PSUM") as ppd,
            tc.tile_pool(name="ppw", bufs=1, space="PSUM") as ppw,
        ):
            # PE warmup: keep the tensor engine busy while the head DMAs land
            # so the HAM clock-gate reaches 2.4 GHz before the real matmuls.
            wtile = pwrm.tile([P, 512], bf16)
            nc.vector.memset(wtile[:], 0.0)
            pwm = ppw.tile([P, 512], f32)
            for w in range(NWARM):
                nc.tensor.matmul(pwm[:], wtile[:, :P], wtile[:],
                                 start=(w == 0), stop=(w == NWARM - 1))

            xt = px.tile([P, HO, C], bf16)
            wgu_t = [pwgu.tile([P, 2, HO, P], bf16, name=f"wgu{j}", tag="wgu")
                     for j in range(IO)]
            wd_t = [pwd.tile([P, WDQ, IO, P], bf16, name=f"wd{q}", tag="wd")
                    for q in range(HO // WDQ)]

            # Input DMAs: ONE ring (SP), consumption order, 0.5-1MB each.
            XH = HO // 2
            nc.sync.dma_start(xt[:, 0:XH, :], xT[:, 0:XH, :])
            nc.sync.dma_start(wgu_t[0][:], wgu[0])
            nc.sync.dma_start(xt[:, XH:HO, :], xT[:, XH:HO, :])
            for j in range(1, IO):
                nc.sync.dma_start(wgu_t[j][:], wgu[j])
            for q in range(HO // WDQ):
                nc.sync.dma_start(wd_t[q][:], wd[q])

            # Phase 1: mid[j] = silu(x @ Wg_j) * (x @ Wu_j) in [I, C] layout.
            mids = []
            for j in range(IO):
                pg = ppg.tile([P, C], f32)
                pu = ppu.tile([P, C], f32)
                for h in range(HO):
                    nc.tensor.matmul(
                        pg[:], wgu_t[j][:, 0, h, :], xt[:, h, :],
                        start=(h == 0), stop=(h == HO - 1),
                    )
                for h in range(HO):
                    nc.tensor.matmul(
                        pu[:], wgu_t[j][:, 1, h, :], xt[:, h, :],
                        start=(h == 0), stop=(h == HO - 1),
                    )
                sg = psg.tile([P, C], f32)
                nc.scalar.activation(sg[:], pg[:], SILU)
                mid = pmid.tile([P, C], bf16)
                nc.vector.tensor_mul(out=mid[:], in0=sg[:], in1=pu[:])
                mids.append(mid)

            # Phase 2: y[k] = sum_j Wd[j, k].T @ mid[j] in [H, C] layout.
            for k in range(HO):
                pd = ppd.tile([P, C], f32)
                for j in range(IO):
                    nc.tensor.matmul(
                        pd[:], wd_t[k // WDQ][:, k % WDQ, j, :], mids[j][:],
                        start=(j == 0), stop=(j == IO - 1),
                    )
                ty = py.tile([P, C], bf16)
                nc.vector.tensor_copy(out=ty[:], in_=pd[:])
                nc.scalar.dma_start(y[k * P:(k + 1) * P, :], ty[:])

    nc.compile()
    return nc


def _route(x, router_w, corr_bias):
    """fp64 router: returns (topk_idx [T,K], topk_w [T,K])."""
    xl = x.astype(np.float64)
    logits = xl @ router_w.astype(np.float64).T
    logits -= logits.max(axis=1, keepdims=True)
    p = np.exp(logits)
    p /= p.sum(axis=1, keepdims=True)
    sel = p + corr_bias.astype(np.float64)
    topk_idx = np.argsort(-sel, axis=1, kind="stable")[:, :TOPK]
    topk_w = np.take_along_axis(p, topk_idx, axis=1) * ROUTED_SCALING
    return topk_idx, topk_w


def kernel(hidden_states, router_w, corr_bias, w_gate, w_up, w_down):
    global _PROGRAM, LAST_RESULTS
    x = np.asarray(hidden_states, dtype=np.float32)
    router_w = np.asarray(router_w, dtype=np.float32)
    corr_bias = np.asarray(corr_bias, dtype=np.float32)
    w_gate = np.asarray(w_gate, dtype=np.float32)
    w_up = np.asarray(w_up, dtype=np.float32)
    w_down = np.asarray(w_down, dtype=np.float32)

    topk_idx, topk_w = _route(x, router_w, corr_bias)
    routed = topk_idx < E
    zero_w = (topk_w * (~routed)).sum(axis=1)  # [T] fp64

    bf = ml_dtypes.bfloat16
    x16 = x.astype(bf)

    # Dispatch: token list + gate weight per expert; overflow beyond C
    # falls back to an exact host computation.
    idx_list, w_list, overflow = [], [], []
    for e in range(E):
        toks, kpos = np.nonzero(topk_idx == e)
        we = topk_w[toks, kpos]
        if len(toks) > C:
            overflow.append((e, toks[C:], we[C:]))
            toks, we = toks[:C], we[:C]
        idx_list.append(toks)
        w_list.append(we)

    in_maps = []
    for e in range(E):
        toks = idx_list[e]
        n = len(toks)
        xg = np.zeros((C, H), dtype=bf)
        xg[:n] = x16[toks]
        xTd = np.ascontiguousarray(
            xg.T.reshape(HO, P, C).transpose(1, 0, 2))
        wgd = w_gate[e].astype(bf).reshape(HO, P, IO, P)
        wud = w_up[e].astype(bf).reshape(HO, P, IO, P)
        # wgu[j, p, g, ho, c]: g=0 → wg, g=1 → wu
        wgud = np.ascontiguousarray(
            np.stack([wgd, wud], axis=0).transpose(3, 2, 0, 1, 4))
        wdd = np.ascontiguousarray(
            w_down[e].astype(bf).reshape(IO, P, HO // WDQ, WDQ, P)
            .transpose(2, 1, 3, 0, 4))
        in_maps.append({"xT": xTd, "wgu": wgud, "wd": wdd})

    if _PROGRAM is None:
        _PROGRAM = _build_program()

    from concourse.bass_utils import run_bass_kernel_spmd

    kw = {}
    if os.environ.get("MOE_KERNEL_TRACE", "") == "1":
        kw = dict(trace=True, trace_cores=list(range(N_CORES)))
    res = run_bass_kernel_spmd(
        _PROGRAM, in_maps, core_ids=list(range(N_CORES)), **kw)
    LAST_RESULTS = res

    out = np.zeros((T, H), dtype=np.float64)
    for e in range(E):
        n = len(idx_list[e])
        if n:
            ye = res.results[e]["y"]  # [H, C] bf16
            out[idx_list[e]] += (w_list[e][:, None]
                                 * ye[:, :n].T.astype(np.float64))
    for e, toks, ws in overflow:
        xt = x[toks].astype(np.float64)
        g = xt @ w_gate[e].astype(np.float64)
        u = xt @ w_up[e].astype(np.float64)
        mid = (g / (1.0 + np.exp(-g))) * u
        out[toks] += ws[:, None] * (mid @ w_down[e].astype(np.float64))
    out += zero_w[:, None] * x.astype(np.float64)
    return out.astype(np.float32)



# revision 2
# speedup vs baseline: 1.0268x; 1.0268x over previous
"""LongcatMoe Trainium2 kernel — expert-parallel sparse MoE across 8 NeuronCores.

Strategy (expert-parallel, per the sharding hint):
  - Host computes the tiny router (fp64 softmax/top-k) and dispatches tokens
    by top-k expert id: core e receives the tokens routed to expert e (padded
    to capacity C=256, capacity factor 1.0; overflow falls back to an exact
    host computation), plus expert e's weights.
  - Each core runs the silu-gated MLP for its expert on its token block.
  - Host combines: out[tok] += gate_weight * y, plus the zero-expert
    (identity) term zero_w[t] * x[t].

v2 kernel: fp8 DoubleRow phase 1 + bf16 phase 2.
  Phase 1 (gate/up) runs "flipped": the stationary PE operand is an fp8
  x-block [128, 2, 128] (DoubleRow packs K=256 contraction rows), the moving
  operand streams fp8 gate/up weights [128, 2, 512] (FD=512), producing
  mid in [C, I] layout in PSUM at ~2 rows/cycle. Scales keep the values in
  e4m3's normal range and are exact powers of two:
      x *= 2^4,  w_gate/w_up *= 2^7   (fp8)
      silu via activation(scale=2^-11) recovers true gate
      the up-path 2^11 factor is folded into w_down (*= 2^-11, bf16, exact)
  mid [C, I] is flipped to [I, C] by DMA XBAR transposes (InstDmaTransposeAnt,
  ~0.5us per unit on the ACT ring, off the PE critical path).
  Phase 2 (down) is unchanged bf16: stationary w_down blocks, stream mid
  [128, 256], fp32 PSUM, y output cast bf16.

DMA design: all inputs ride the SP ring in exact consumption order with
>=512KB transfers; mid transposes and the y output ride the ACT ring.

Host-side layouts (per-partition contiguous for every device DMA):
  x8   [128, KH, 2, C]  fp8     x8[p,kh,o,c]     = x[idx[c], kh*256+o*128+p]*2^4
  wgu8 [2, 4, 128, 2, 2, 2, 512] fp8
                        wgu8[ih,khp,p,khi,g,o,n] = w{g}[(2khp+khi)*256+o*128+p,
                                                        ih*512+n]*2^7
  wd   [HO/4, 128, 4, IO, 128] bf16  wd[q,p,i,io,c] = w_down[io*128+p,
                                                        (4q+i)*128+c]*2^-11
  y    [H, C] bf16 output
"""

import os

import numpy as np
import ml_dtypes

T, H, I, E, Z, TOPK = 1024, 2048, 1024, 8, 8, 4
ROUTED_SCALING = 1.0
N_CORES = 8
P = 128
HO = H // P   # 16
IO = I // P   # 8
KH = H // 256  # 8 DoubleRow k-groups
C = 256       # per-expert token capacity (capacity factor 1.0; overflow → host)
WDQ = 4       # wd k-slices per DMA transfer
NWARM = 14    # PE warmup matmuls (bridge HAM until first real matmul)

XS = 2.0 ** 4   # x fp8 pre-scale
WS = 2.0 ** 7   # w_gate/w_up fp8 pre-scale
DS = 2.0 ** -11  # inverse of XS*WS; silu scale and w_down fold

_PROGRAM = None
LAST_RESULTS = None  # BassKernelResults of the most recent run (for test harness)


def _build_program():
    import concourse.mybir as mybir
    import concourse.tile as tile
    from concourse import bacc

    f32 = mybir.dt.float32
    bf16 = mybir.dt.bfloat16
    fp8 = mybir.dt.float8e4
    SILU = mybir.ActivationFunctionType.Silu
    DR = mybir.MatmulPerfMode.DoubleRow

    nc = bacc.Bacc(
        "TRN2",
        target_bir_lowering=False,
        debug=False,
        enable_asserts=False,
        num_devices=N_CORES,
    )
    x8 = nc.dram_tensor("x8", [P, KH, 2, C], fp8, kind="ExternalInput").ap()
    wgu8 = nc.dram_tensor("wgu8", [2, KH // 2, P, 2, 2, 2, 512], fp8,
                          kind="ExternalInput").ap()
    wd = nc.dram_tensor("wd", [HO // WDQ, P, WDQ, IO, P], bf16,
                        kind="ExternalInput").ap()
    y = nc.dram_tensor("y", [H, C], bf16, kind="ExternalOutput").ap()

    with tile.TileContext(nc) as tc:
        with (
            tc.tile_pool(name="px", bufs=1) as px,
            tc.tile_pool(name="pwgu", bufs=KH) as pwgu,
            tc.tile_pool(name="pwd", bufs=HO // WDQ) as pwd,
            tc.tile_pool(name="pmidc", bufs=4) as pmidc,
            tc.tile_pool(name="pmidi", bufs=1) as pmidi,
            tc.tile_pool(name="psg", bufs=2) as psg,
            tc.tile_pool(name="py", bufs=6) as py,
            tc.tile_pool(name="pwrm", bufs=1) as pwrm,
            tc.tile_pool(name="ppg", bufs=2, space="PSUM") as ppg,
            tc.tile_pool(name="ppu", bufs=2, space="PSUM") as ppu,
            tc.tile_pool(name="ppd", bufs=3, space="PSUM") as ppd,
            tc.tile_pool(name="ppw", bufs=1, space="PSUM") as ppw,
        ):
            # PE warmup: keep the tensor engine busy while the head DMAs land
            # so the HAM clock-gate reaches 2.4 GHz by the first real matmul.
            wtile = pwrm.tile([P, C], bf16)
            nc.vector.memset(wtile[:], 0.0)
            pwm = ppw.tile([P, C], f32)
            for w in range(NWARM):
                nc.tensor.matmul(pwm[:], wtile[:, :P], wtile[:],
                                 start=(w == 0), stop=(w == NWARM - 1))

            xt = px.tile([P, KH, 2, C], fp8)
            wgu_t = [pwgu.tile([P, 2, 2, 2, 512], fp8, name=f"wgu{t}",
                               tag="wgu") for t in range(KH)]
            wd_t = [pwd.tile([P, WDQ, IO, P], bf16, name=f"wd{q}", tag="wd")
                    for q in range(HO // WDQ)]
            mid_i = pmidi.tile([P, IO, C], bf16)

            # Input DMAs: ONE ring (SP), consumption order, >=512KB each.
            nc.sync.dma_start(xt[:], x8[:])
            for ih in range(2):
                for khp in range(KH // 2):
                    nc.sync.dma_start(wgu_t[ih * (KH // 2) + khp][:],
                                      wgu8[ih, khp])
            for q in range(HO // WDQ):
                nc.sync.dma_start(wd_t[q][:], wd[q])

            # Phase 1 (flipped, fp8 DoubleRow): for unit (cb, ih), psum
            # [c=128, i=512] accumulates over KH k-groups of 256 h-rows.
            # c-inner unit order so each wgu half is consumed by two units
            # while DMA fetches the next half.
            for ih in range(2):
                for cb in range(2):
                    pg = ppg.tile([P, 512], f32)
                    pu = ppu.tile([P, 512], f32)
                    for kh in range(KH):
                        wt = wgu_t[ih * (KH // 2) + kh // 2]
                        khi = kh % 2
                        xst = xt[:, kh, :, cb * P:(cb + 1) * P]
                        nc.tensor.matmul(
                            pg[:], xst, wt[:, khi, 0, :, :],
                            start=(kh == 0), stop=(kh == KH - 1),
                            perf_mode=DR,
                        )
                        nc.tensor.matmul(
                            pu[:], xst, wt[:, khi, 1, :, :],
                            start=(kh == 0), stop=(kh == KH - 1),
                            perf_mode=DR,
                        )
                    sg = psg.tile([P, 512], f32)
                    nc.scalar.activation(sg[:], pg[:], SILU, scale=DS)
                    midc = pmidc.tile([P, 512], bf16)
                    nc.vector.tensor_mul(out=midc[:], in0=sg[:], in1=pu[:])
                    # Flip [c,i]->[i,c] via DMA XBAR transpose on the ACT ring.
                    nc.scalar.dma_start_transpose(
                        mid_i[:, ih * 4:(ih + 1) * 4, cb * P:(cb + 1) * P],
                        midc[:])

            # Phase 2 (bf16): y[k] = sum_j Wd[j, k].T @ mid[j] in [H, C].
            for k in range(HO):
                pd = ppd.tile([P, C], f32)
                for j in range(IO):
                    nc.tensor.matmul(
                        pd[:], wd_t[k // WDQ][:, k % WDQ, j, :],
                        mid_i[:, j, :],
                        start=(j == 0), stop=(j == IO - 1),
                    )
                ty = py.tile([P, C], bf16)
                nc.vector.tensor_copy(out=ty[:], in_=pd[:])
                nc.scalar.dma_start(y[k * P:(k + 1) * P, :], ty[:])

    nc.compile()
    return nc


def _route(x, router_w, corr_bias):
    """fp64 router: returns (topk_idx [T,K], topk_w [T,K])."""
    xl = x.astype(np.float64)
    logits = xl @ router_w.astype(np.float64).T
    logits -= logits.max(axis=1, keepdims=True)
    p = np.exp(logits)
    p /= p.sum(axis=1, keepdims=True)
    sel = p + corr_bias.astype(np.float64)
    topk_idx = np.argsort(-sel, axis=1, kind="stable")[:, :TOPK]
    topk_w = np.take_along_axis(p, topk_idx, axis=1) * ROUTED_SCALING
    return topk_idx, topk_w


def kernel(hidden_states, router_w, corr_bias, w_gate, w_up, w_down):
    global _PROGRAM, LAST_RESULTS
    x = np.asarray(hidden_states, dtype=np.float32)
    router_w = np.asarray(router_w, dtype=np.float32)
    corr_bias = np.asarray(corr_bias, dtype=np.float32)
    w_gate = np.asarray(w_gate, dtype=np.float32)
    w_up = np.asarray(w_up, dtype=np.float32)
    w_down = np.asarray(w_down, dtype=np.float32)

    topk_idx, topk_w = _route(x, router_w, corr_bias)
    routed = topk_idx < E
    zero_w = (topk_w * (~routed)).sum(axis=1)  # [T] fp64

    bf = ml_dtypes.bfloat16
    e4 = ml_dtypes.float8_e4m3

    # Dispatch: token list + gate weight per expert; overflow beyond C
    # falls back to an exact host computation.
    idx_list, w_list, overflow = [], [], []
    for e in range(E):
        toks, kpos = np.nonzero(topk_idx == e)
        we = topk_w[toks, kpos]
        if len(toks) > C:
            overflow.append((e, toks[C:], we[C:]))
            toks, we = toks[:C], we[:C]
        idx_list.append(toks)
        w_list.append(we)

    in_maps = []
    for e in range(E):
        toks = idx_list[e]
        n = len(toks)
        xg = np.zeros((C, H), dtype=np.float32)
        xg[:n] = x[toks]
        # x8[p, kh, o, c] = x[c, kh*256+o*128+p] * XS
        x8d = np.ascontiguousarray(
            (xg * XS).astype(e4).reshape(C, KH, 2, P).transpose(3, 1, 2, 0))
        # wgu8[ih, khp, p, khi, g, o, n]
        #   = w{g}[(2khp+khi)*256+o*128+p, ih*512+n] * WS
        wg8 = (w_gate[e] * WS).astype(e4)
        wu8 = (w_up[e] * WS).astype(e4)
        # [g, H, I] -> [g, khp, khi, o, p, ih, n]
        wgu_s = np.stack([wg8, wu8], axis=0).reshape(
            2, KH // 2, 2, 2, P, 2, 512)
        wgud = np.ascontiguousarray(wgu_s.transpose(5, 1, 4, 2, 0, 3, 6))
        wdd = np.ascontiguousarray(
            (w_down[e] * DS).astype(bf).reshape(IO, P, HO // WDQ, WDQ, P)
            .transpose(2, 1, 3, 0, 4))
        in_maps.append({"x8": x8d, "wgu8": wgud, "wd": wdd})

    if _PROGRAM is None:
        _PROGRAM = _build_program()

    from concourse.bass_utils import run_bass_kernel_spmd

    kw = {}
    if os.environ.get("MOE_KERNEL_TRACE", "") == "1":
        kw = dict(trace=True, trace_cores=list(range(N_CORES)))
    res = run_bass_kernel_spmd(
        _PROGRAM, in_maps, core_ids=list(range(N_CORES)), **kw)
    LAST_RESULTS = res

    out = np.zeros((T, H), dtype=np.float64)
    for e in range(E):
        n = len(idx_list[e])
        if n:
            ye = res.results[e]["y"]  # [H, C] bf16
            out[idx_list[e]] += (w_list[e][:, None]
                                 * ye[:, :n].T.astype(np.float64))
    for e, toks, ws in overflow:
        xt = x[toks].astype(np.float64)
        g = xt @ w_gate[e].astype(np.float64)
        u = xt @ w_up[e].astype(np.float64)
        mid = (g / (1.0 + np.exp(-g))) * u
        out[toks] += ws[:, None] * (mid @ w_down[e].astype(np.float64))
    out += zero_w[:, None] * x.astype(np.float64)
    return out.astype(np.float32)


# revision 7
# speedup vs baseline: 1.1164x; 1.0873x over previous
"""LongcatMoe Trainium2 kernel — expert-parallel sparse MoE across 8 NeuronCores.

Strategy (expert-parallel, per the sharding hint):
  - Host computes the tiny router (fp64 softmax/top-k) and dispatches tokens
    by top-k expert id: core e receives the tokens routed to expert e (padded
    to capacity C=256, capacity factor 1.0; overflow falls back to an exact
    host computation), plus expert e's weights.
  - Each core runs the silu-gated MLP for its expert on its token block.
  - Host combines: out[tok] += gate_weight * y, plus the zero-expert
    (identity) term zero_w[t] * x[t].

v3 kernel: fp8 DoubleRow phase 1 + bf16 phase 2.
  Phase 1 (gate/up) runs "flipped": the stationary PE operand is an fp8
  x-block [128, 2, 128] (DoubleRow packs K=256 contraction rows), the moving
  operand streams fp8 gate/up weights [128, 2, 512] (FD=512) at 2 rows/cycle
  (measured 216 ns/MM warm = 2x the bf16 rate), producing mid in [C, I]
  layout in PSUM. Scales keep values in e4m3's normal range, all exact
  powers of two:
      x *= 2^4,  w_gate/w_up *= 2^7   (fp8)
      silu via activation(scale=2^-11) recovers the true gate
      the up-path 2^11 factor is folded into w_down (*= 2^-11, bf16, exact)
  mid [C, I] is flipped to [I, C] by DMA XBAR transposes on the ACT ring
  (~0.5us each, off the PE critical path).
  Phase 2 (down) is bf16: stationary w_down blocks, stream mid [128, 256],
  fp32 PSUM, y pairs cast bf16 and written out on the ACT ring.

DMA design (from v2 trace analysis): the HWDGE queues throttle when >8-ish
transfers share the global DMA-semaphore pool (await-space waits chained
across queues serialized w_down behind the mid transposes, starving phase 2
and oscillating the HAM clock gate). So: few, big transfers. SP ring carries
the weight stream in exact consumption order (wguA/B per i-half, then wd
halves); the ACT ring carries x8 (overlapped with the wgu head), the 4 mid
transposes, and 8 paired y writes.

Host-side layouts (per-partition contiguous for every device DMA):
  x8   [128, KH, 2, C]  fp8     x8[p,kh,o,c]   = x[idx[c], kh*256+o*128+p]*2^4
  wgu8 [2, 128, KH, 2, 2, 512] fp8
                        wgu8[ih,p,kh,g,o,n] = w{g}[kh*256+o*128+p, ih*512+n]*2^7
  wd   [2, 128, 8, IO, 128] bf16  wd[s,p,k,j,c] = w_down[j*128+p,
                                                   (8s+k)*128+c]*2^-11
  y    [HO/2, 128, 2, C] bf16 output; host reassembles [H, C]
"""

import os

import numpy as np
import ml_dtypes

T, H, I, E, Z, TOPK = 1024, 2048, 1024, 8, 8, 4
ROUTED_SCALING = 1.0
N_CORES = 8
P = 128
HO = H // P   # 16
IO = I // P   # 8
KH = H // 256  # 8 DoubleRow k-groups
C = 256       # per-expert token capacity (capacity factor 1.0; overflow → host)
KA = 2        # kh's in the wgu head chunk (quick phase-1 start)
NWARM = 16    # PE warmup matmuls (bridge HAM until first real matmul)

XS = 2.0 ** 4   # x fp8 pre-scale
WS = 2.0 ** 7   # w_gate/w_up fp8 pre-scale
DS = 2.0 ** -11  # inverse of XS*WS; silu scale and w_down fold

_PROGRAM = None
LAST_RESULTS = None  # BassKernelResults of the most recent run (for test harness)


def _build_program():
    import concourse.mybir as mybir
    import concourse.tile as tile
    from concourse import bacc

    f32 = mybir.dt.float32
    bf16 = mybir.dt.bfloat16
    fp8 = mybir.dt.float8e4
    SILU = mybir.ActivationFunctionType.Silu
    DR = mybir.MatmulPerfMode.DoubleRow

    nc = bacc.Bacc(
        "TRN2",
        target_bir_lowering=False,
        debug=False,
        enable_asserts=False,
        num_devices=N_CORES,
    )
    x8 = nc.dram_tensor("x8", [P, KH, 2, C], fp8, kind="ExternalInput").ap()
    wgu8 = nc.dram_tensor("wgu8", [2, P, KH, 2, 2, 512], fp8,
                          kind="ExternalInput").ap()
    wd = nc.dram_tensor("wd", [2, P, HO // 2, IO, P], bf16,
                        kind="ExternalInput").ap()
    y = nc.dram_tensor("y", [HO // 2, P, 2, C], bf16, kind="ExternalOutput").ap()

    with tile.TileContext(nc) as tc:
        with (
            tc.tile_pool(name="px", bufs=1) as px,
            tc.tile_pool(name="pwgu", bufs=4) as pwgu,
            tc.tile_pool(name="pwd", bufs=2) as pwd,
            tc.tile_pool(name="pmidc", bufs=4) as pmidc,
            tc.tile_pool(name="pmidi", bufs=1) as pmidi,
            tc.tile_pool(name="psg", bufs=2) as psg,
            tc.tile_pool(name="py", bufs=4) as py,
            tc.tile_pool(name="pwrm", bufs=1) as pwrm,
            tc.tile_pool(name="ppg", bufs=2, space="PSUM") as ppg,
            tc.tile_pool(name="ppu", bufs=2, space="PSUM") as ppu,
            tc.tile_pool(name="ppd", bufs=3, space="PSUM") as ppd,
            tc.tile_pool(name="ppw", bufs=1, space="PSUM") as ppw,
        ):
            # PE warmup: keep the tensor engine busy while the head DMAs land
            # so the HAM clock-gate reaches 2.4 GHz by the first real matmul.
            wtile = pwrm.tile([P, C], bf16)
            nc.vector.memset(wtile[:], 0.0)
            pwm = ppw.tile([P, C], f32)
            for w in range(NWARM):
                nc.tensor.matmul(pwm[:], wtile[:, :P], wtile[:],
                                 start=(w == 0), stop=(w == NWARM - 1))

            xt = px.tile([P, KH, 2, C], fp8)
            # wgu SBUF tiles: per ih a head chunk (KA kh's) and the tail.
            wgu_t = []  # wgu_t[ih] = (head_tile, tail_tile)
            for ih in range(2):
                a = pwgu.tile([P, KA, 2, 2, 512], fp8, name=f"wguA{ih}",
                              tag="wgu")
                b = pwgu.tile([P, KH - KA, 2, 2, 512], fp8, name=f"wguB{ih}",
                              tag="wgu")
                wgu_t.append((a, b))
            wd_t = [pwd.tile([P, HO // 2, IO, P], bf16, name=f"wd{s}",
                             tag="wd") for s in range(2)]
            mid_i = pmidi.tile([P, IO, C], bf16)

            # Input DMAs. SP ring: the weight stream in consumption order,
            # 6 transfers so the HWDGE never hits await-space throttling.
            # ACT ring: x8 (overlaps the wgu head on SP).
            nc.scalar.dma_start(xt[:], x8[:])
            for ih in range(2):
                nc.sync.dma_start(wgu_t[ih][0][:], wgu8[ih][:, 0:KA])
                nc.sync.dma_start(wgu_t[ih][1][:], wgu8[ih][:, KA:KH])
            for s in range(2):
                nc.sync.dma_start(wd_t[s][:], wd[s])

            # Phase 1 (flipped, fp8 DoubleRow): for unit (cb, ih), psum
            # [c=128, i=512] accumulates over KH k-groups of 256 h-rows.
            # c-inner unit order so each wgu half is consumed by two units
            # while DMA fetches the next half.
            for ih in range(2):
                for cb in range(2):
                    pg = ppg.tile([P, 512], f32)
                    pu = ppu.tile([P, 512], f32)
                    for kh in range(KH):
                        wt = wgu_t[ih][0] if kh < KA else wgu_t[ih][1]
                        ko = kh if kh < KA else kh - KA
                        xst = xt[:, kh, :, cb * P:(cb + 1) * P]
                        nc.tensor.matmul(
                            pg[:], xst, wt[:, ko, 0, :, :],
                            start=(kh == 0), stop=(kh == KH - 1),
                            perf_mode=DR,
                        )
                        nc.tensor.matmul(
                            pu[:], xst, wt[:, ko, 1, :, :],
                            start=(kh == 0), stop=(kh == KH - 1),
                            perf_mode=DR,
                        )
                    sg = psg.tile([P, 512], f32)
                    nc.scalar.activation(sg[:], pg[:], SILU, scale=DS)
                    midc = pmidc.tile([P, 512], bf16)
                    nc.vector.tensor_mul(out=midc[:], in0=sg[:], in1=pu[:])
                    # Flip [c,i]->[i,c] via DMA XBAR transpose on the ACT ring.
                    nc.scalar.dma_start_transpose(
                        mid_i[:, ih * 4:(ih + 1) * 4, cb * P:(cb + 1) * P],
                        midc[:])

            # Phase 2 (bf16): y[k] = sum_j Wd[j, k].T @ mid[j] in [H, C];
            # y tiles written out in pairs to halve the DMA count.
            for kq in range(HO // 2):
                ty = py.tile([P, 2, C], bf16)
                for half in range(2):
                    k = kq * 2 + half
                    pd = ppd.tile([P, C], f32)
                    for j in range(IO):
                        nc.tensor.matmul(
                            pd[:], wd_t[k // (HO // 2)][:, k % (HO // 2), j, :],
                            mid_i[:, j, :],
                            start=(j == 0), stop=(j == IO - 1),
                        )
                    nc.vector.tensor_copy(out=ty[:, half, :], in_=pd[:])
                nc.scalar.dma_start(y[kq], ty[:])

    nc.compile()
    return nc


def _route(x, router_w, corr_bias):
    """fp64 router: returns (topk_idx [T,K], topk_w [T,K])."""
    xl = x.astype(np.float64)
    logits = xl @ router_w.astype(np.float64).T
    logits -= logits.max(axis=1, keepdims=True)
    p = np.exp(logits)
    p /= p.sum(axis=1, keepdims=True)
    sel = p + corr_bias.astype(np.float64)
    topk_idx = np.argsort(-sel, axis=1, kind="stable")[:, :TOPK]
    topk_w = np.take_along_axis(p, topk_idx, axis=1) * ROUTED_SCALING
    return topk_idx, topk_w


def kernel(hidden_states, router_w, corr_bias, w_gate, w_up, w_down):
    global _PROGRAM, LAST_RESULTS
    x = np.asarray(hidden_states, dtype=np.float32)
    router_w = np.asarray(router_w, dtype=np.float32)
    corr_bias = np.asarray(corr_bias, dtype=np.float32)
    w_gate = np.asarray(w_gate, dtype=np.float32)
    w_up = np.asarray(w_up, dtype=np.float32)
    w_down = np.asarray(w_down, dtype=np.float32)

    topk_idx, topk_w = _route(x, router_w, corr_bias)
    routed = topk_idx < E
    zero_w = (topk_w * (~routed)).sum(axis=1)  # [T] fp64

    bf = ml_dtypes.bfloat16
    e4 = ml_dtypes.float8_e4m3

    # Dispatch: token list + gate weight per expert; overflow beyond C
    # falls back to an exact host computation.
    idx_list, w_list, overflow = [], [], []
    for e in range(E):
        toks, kpos = np.nonzero(topk_idx == e)
        we = topk_w[toks, kpos]
        if len(toks) > C:
            overflow.append((e, toks[C:], we[C:]))
            toks, we = toks[:C], we[:C]
        idx_list.append(toks)
        w_list.append(we)

    in_maps = []
    for e in range(E):
        toks = idx_list[e]
        n = len(toks)
        xg = np.zeros((C, H), dtype=np.float32)
        xg[:n] = x[toks]
        # x8[p, kh, o, c] = x[c, kh*256+o*128+p] * XS
        x8d = np.ascontiguousarray(
            (xg * XS).astype(e4).reshape(C, KH, 2, P).transpose(3, 1, 2, 0))
        # wgu8[ih, p, kh, g, o, n] = w{g}[kh*256+o*128+p, ih*512+n] * WS
        wg8 = (w_gate[e] * WS).astype(e4)
        wu8 = (w_up[e] * WS).astype(e4)
        # [g, H, I] -> [g, kh, o, p, ih, n] -> [ih, p, kh, g, o, n]
        wgu_s = np.stack([wg8, wu8], axis=0).reshape(2, KH, 2, P, 2, 512)
        wgud = np.ascontiguousarray(wgu_s.transpose(4, 3, 1, 0, 2, 5))
        # wd[s, p, k, j, c] = w_down[j*128+p, (8s+k)*128+c] * DS
        wdd = np.ascontiguousarray(
            (w_down[e] * DS).astype(bf).reshape(IO, P, 2, HO // 2, P)
            .transpose(2, 1, 3, 0, 4))
        in_maps.append({"x8": x8d, "wgu8": wgud, "wd": wdd})

    if _PROGRAM is None:
        _PROGRAM = _build_program()

    from concourse.bass_utils import run_bass_kernel_spmd

    kw = {}
    if os.environ.get("MOE_KERNEL_TRACE", "") == "1":
        kw = dict(trace=True, trace_cores=list(range(N_CORES)))
    res = run_bass_kernel_spmd(
        _PROGRAM, in_maps, core_ids=list(range(N_CORES)), **kw)
    LAST_RESULTS = res

    out = np.zeros((T, H), dtype=np.float64)
    for e in range(E):
        n = len(idx_list[e])
        if n:
            yr = res.results[e]["y"]  # [HO//2, P, 2, C] bf16
            ye = yr.transpose(0, 2, 1, 3).reshape(H, C)
            out[idx_list[e]] += (w_list[e][:, None]
                                 * ye[:, :n].T.astype(np.float64))
    for e, toks, ws in overflow:
        xt = x[toks].astype(np.float64)
        g = xt @ w_gate[e].astype(np.float64)
        u = xt @ w_up[e].astype(np.float64)
        mid = (g / (1.0 + np.exp(-g))) * u
        out[toks] += ws[:, None] * (mid @ w_down[e].astype(np.float64))
    out += zero_w[:, None] * x.astype(np.float64)
    return out.astype(np.float32)
